# revision 1
# baseline (speedup 1.0000x reference)
"""NePuEncoder Bass/Tile kernel for 8 Trainium2 NeuronCores.

Sharding: query-parallel. Core c handles batch b=c//4, queries qo=(c%4)*96 ..
qo+96 of that batch. Channel-major layout [128 chan, keys] throughout.

Per-channel softmax attention fully fused in SBUF/PSUM:
  - pairwise trig features via range-reduced Sin (|arg|<=pi), computed once and
    reused across all 3 transformer blocks
  - per-query: 5 bf16 matmuls (trig->hpre, +Wg1@EK accum, logits, trig->pos,
    +I@VK accum), ACT relu (per-query bias) + batched ACT exp, DVE
    scalar_tensor_tensor (pos+QP)*w accumulate (S1) and tensor_scalar
    accumulate (S0 = sum w)
  - one 8-rank AllGather per block carries o=res+f plus per-chunk BN stats;
    BatchNorm applied replicated. MLP blocks and the final FC are computed
    redundantly per core (cheap); core 0's output is returned.
"""
import sys

sys.path.insert(0, "/opt/trn_rl_repo")

import numpy as np
import ml_dtypes

B, N, D, DS, LAT, FD, NF = 2, 384, 128, 3, 256, 1, 2
NB = NF + 1
NQ = 96                  # queries per core
NG = NQ // 4
RELU_ACT_FRAC10 = 3      # m%10 < this -> relu on ACT, else DVE
FREQS = np.linspace(1.0, 32.0, 5).astype(np.float64)
EPS = 1e-5
TWO_PI = float(2 * np.pi)
C_ROUND = float(3 << 22)  # 2^23 + 2^22: fp32 round-to-nearest-even trick

BF = ml_dtypes.bfloat16


def _bf(x):
    return np.ascontiguousarray(np.asarray(x, np.float32).astype(BF))


def _f32(x):
    return np.ascontiguousarray(np.asarray(x, np.float32))


def _wpe_split(Wpe):
    """W_s [128,30] trig cols (sin-sign absorbed for npd = xk - xq), W_d [128,3]."""
    Ws = np.zeros((D, 30), np.float32)
    for j in range(3):
        for t in range(10):
            r = 10 * j + t
            if t < 5:
                Ws[:, r] = -Wpe[:, 3 + 6 * t + j]
            else:
                Ws[:, r] = Wpe[:, 3 + 6 * (t - 5) + 3 + j]
    return Ws, Wpe[:, 0:3].astype(np.float32)


_CACHE = {}


def _build(variant="spmd"):
    if variant in _CACHE:
        return _CACHE[variant]

    import concourse.bacc as bacc
    import concourse.bass as bass
    import concourse.tile as tile
    from concourse import mybir

    F32, BF16 = mybir.dt.float32, mybir.dt.bfloat16
    AF = mybir.ActivationFunctionType
    OP = mybir.AluOpType

    nc = bacc.Bacc(None, target_bir_lowering=False,
                   num_devices=(8 if variant == "spmd" else 1))

    def din(name, shape, dt=BF16):
        return nc.dram_tensor(name, shape, dt, kind="ExternalInput")

    # per-core inputs
    xk_f = din("xk_f", [3, 384], F32)
    s4_f = din("s4_f", [4, 32 * 96], F32)
    xk_b = din("xk_b", [3, 384])
    xq_b = din("xq_b", [3, 96])
    feats_b = din("feats_b", [1, 384])
    feats_q = din("feats_q", [1, 96])
    # replicated inputs
    feats_row = din("feats_row", [1, 768])
    i128 = din("i128", [128, 128])
    L_G1P = din("L_G1P", [NB, 30, 128])
    L_Wg1 = din("L_Wg1", [NB, 128, 128])
    L_Wg2 = din("L_Wg2", [NB, 128, 128])
    L_Ps = din("L_Ps", [NB, 30, 128])
    L_nWk = din("L_nWk", [NB, 128, 128])
    L_Wv = din("L_Wv", [NB, 128, 128])
    L_nPd4 = din("L_nPd4", [NB, 3, 128])
    L_G1Q = din("L_G1Q", [NB, 128, 128])
    L_G1Pd4 = din("L_G1Pd4", [NB, 3, 128])
    L_Pd4 = din("L_Pd4", [NB, 3, 128])
    row_c1 = din("row_c1", [NB, 1, 128])
    row_bpe = din("row_bpe", [NB, 1, 128])
    bg2_v = din("bg2_v", [NB, 128, 1], F32)
    gamma_v = din("gamma_v", [NB, 128, 1], F32)
    beta_v = din("beta_v", [NB, 128, 1], F32)
    L_enc = din("L_enc", [1, 128])
    encb_v = din("encb_v", [128, 1], F32)
    L_em1 = din("L_em1", [NF, 128, 128])
    L_em2 = din("L_em2", [NF, 128, 128])
    em_b1_v = din("em_b1_v", [NF, 128, 1], F32)
    em_b2_v = din("em_b2_v", [NF, 128, 1], F32)
    em_g_v = din("em_g_v", [NF, 128, 1], F32)
    em_be_v = din("em_be_v", [NF, 128, 1], F32)
    L_f1 = din("L_f1", [2, 128, 128])
    f1_b = din("f1_b", [2, 128, 1], F32)
    L_f2 = din("L_f2", [2, 2, 128, 128])
    f2_b = din("f2_b", [2, 128, 1], F32)

    out_d = nc.dram_tensor("out", [2, 256], F32, kind="ExternalOutput")
    dbg = (variant == "dbg")
    if dbg:
        dbg_trig = nc.dram_tensor("dbg_trig", [96, 384], F32, kind="ExternalOutput")
        dbg_EK = nc.dram_tensor("dbg_EK", [128, 384], F32, kind="ExternalOutput")
        dbg_QB = nc.dram_tensor("dbg_QB", [128, 96], F32, kind="ExternalOutput")
        dbg_S0 = nc.dram_tensor("dbg_S0", [128, 96], F32, kind="ExternalOutput")
        dbg_S1 = nc.dram_tensor("dbg_S1", [128, 96], F32, kind="ExternalOutput")
        dbg_f0 = nc.dram_tensor("dbg_f0", [128, 384], F32, kind="ExternalOutput")
    RG = [[0, 1, 2, 3, 4, 5, 6, 7]]

    with tile.TileContext(nc) as tc:
        with (
            tc.tile_pool(name="sing", bufs=1) as sing,
            tc.tile_pool(name="fpool", bufs=2) as fpool,
            tc.tile_pool(name="blk", bufs=2) as blk,
            tc.tile_pool(name="hp", bufs=4) as hp,
            tc.tile_pool(name="wp", bufs=4) as wp,
            tc.tile_pool(name="wide", bufs=2) as wide,
            tc.tile_pool(name="smalls", bufs=4) as smalls,
            tc.tile_pool(name="ps_a", bufs=4, space="PSUM") as ps_a,
            tc.tile_pool(name="ps_b", bufs=2, space="PSUM") as ps_b,
            tc.tile_pool(name="ps_g", bufs=2, space="PSUM") as ps_g,
            tc.tile_pool(name="dram", bufs=1, space="DRAM") as dram,
        ):
            _dmaq = [nc.sync, nc.gpsimd]
            _qi = [0]

            def _q():
                e = _dmaq[_qi[0] % len(_dmaq)]
                _qi[0] += 1
                return e

            def load(src, shape, dt=BF16, pool=sing, tag=None):
                t = pool.tile(shape, dt, tag=tag, name=tag or "ld")
                _q().dma_start(out=t, in_=src)
                return t

            def loadfam(srcT, nblk, tag):
                t = sing.tile([128, nblk * 128], BF16, tag=tag, name=tag)
                ap = srcT[:]
                s = bass.AP(tensor=ap.tensor, offset=ap.offset,
                            ap=[[128, 128], [128 * 128, nblk], [1, 128]])
                _q().dma_start(out=t.rearrange("p (i c) -> p i c", i=nblk), in_=s)
                return [t[:, i * 128:(i + 1) * 128] for i in range(nblk)]

            def load3(srcs, tag):
                t = sing.tile([128, 128], BF16, tag=tag, name=tag)
                for s3 in range(3):
                    _q().dma_start(out=t[32 * s3:32 * s3 + 30, :], in_=srcs)
                return t

            s4_sb = load(s4_f[:].rearrange("a (c q) -> a c q", c=32),
                         [4, 32, 96], F32, tag="s4")

            def load_weights():
                o = {}
                o["xkb"] = load(xk_b[:], [3, 384], tag="sxkb")
                o["xqb"] = load(xq_b[:], [3, 96], tag="sxqb")
                o["featsb"] = load(feats_b[:], [1, 384], tag="sfb")
                o["featsq"] = load(feats_q[:], [1, 96], tag="sfq")
                o["feats"] = load(feats_row[:], [1, 768], tag="sfr")
                o["i128"] = load(i128[:], [128, 128], tag="si128")
                o["G1P"] = [load3(L_G1P[i], f"g1p{i}") for i in range(NB)]
                o["Ps"] = [load3(L_Ps[i], f"ps{i}") for i in range(NB)]
                o["Wg1"] = loadfam(L_Wg1, NB, "wg1")
                o["Wg2"] = loadfam(L_Wg2, NB, "wg2")
                o["nWk"] = loadfam(L_nWk, NB, "nwk")
                o["Wv"] = loadfam(L_Wv, NB, "wv")
                o["G1Q"] = loadfam(L_G1Q, NB, "g1q")
                o["nPd4"] = [load(L_nPd4[i], [3, 128], tag=f"npd{i}") for i in range(NB)]
                o["G1Pd4"] = [load(L_G1Pd4[i], [3, 128], tag=f"g1pd{i}") for i in range(NB)]
                o["Pd4"] = [load(L_Pd4[i], [3, 128], tag=f"pd4{i}") for i in range(NB)]
                o["c1"] = [load(row_c1[i], [1, 128], tag=f"c1{i}") for i in range(NB)]
                o["bpe"] = [load(row_bpe[i], [1, 128], tag=f"bpe{i}") for i in range(NB)]
                o["bg2"] = [load(bg2_v[i], [128, 1], F32, tag=f"bg2{i}") for i in range(NB)]
                o["gam"] = [load(gamma_v[i], [128, 1], F32, tag=f"gam{i}") for i in range(NB)]
                o["bet"] = [load(beta_v[i], [128, 1], F32, tag=f"bet{i}") for i in range(NB)]
                o["enc"] = load(L_enc[:], [1, 128], tag="enc")
                o["encb"] = load(encb_v[:], [128, 1], F32, tag="encb")
                o["em1"] = loadfam(L_em1, NF, "em1")
                o["em2"] = loadfam(L_em2, NF, "em2")
                o["emb1"] = [load(em_b1_v[j], [128, 1], F32, tag=f"emb1{j}") for j in range(NF)]
                o["emb2"] = [load(em_b2_v[j], [128, 1], F32, tag=f"emb2{j}") for j in range(NF)]
                o["emg"] = [load(em_g_v[j], [128, 1], F32, tag=f"emg{j}") for j in range(NF)]
                o["embe"] = [load(em_be_v[j], [128, 1], F32, tag=f"embe{j}") for j in range(NF)]
                o["f1"] = loadfam(L_f1, 2, "f1")
                o["f1b"] = [load(f1_b[h], [128, 1], F32, tag=f"f1b{h}") for h in range(2)]
                o["f2"] = [[load(L_f2[h, k], [128, 128], tag=f"f2{h}{k}") for k in range(2)]
                           for h in range(2)]
                o["f2b"] = [load(f2_b[h], [128, 1], F32, tag=f"f2b{h}") for h in range(2)]
                return o

            ones96 = sing.tile([1, 96], BF16, tag="ones96")
            nc.vector.memset(ones96, 1.0)
            eps_t = sing.tile([128, 1], F32, tag="epst")
            nc.vector.memset(eps_t, EPS)
            dumA = sing.tile([128, 1], BF16, tag="dumA")
            dumB = sing.tile([128, 1], BF16, tag="dumB")

            trigc = [sing.tile([96, 384], BF16, tag=f"trig{c}", name=f"trig{c}")
                     for c in range(32)]

            # ---------- stage 1: trig features via S-matmul + range-reduced Sin ----
            # r[row,k] = s*(xk[j,k]-xq[q,j]) + off  computed as fp32 matmul
            # S4 @ [xk;ones];  n = (r+C)-C (round);  -frac = (n-C)-r;
            # trig = sin(-2pi * -frac) -> bf16, query q_rel at partitions 32*q_rel.
            with tc.tile_pool(name="st1", bufs=3) as st1:
                xko = sing.tile([4, 384], F32, tag="xko")
                nc.vector.memset(xko, 1.0)
                nc.sync.dma_start(out=xko[0:3, :], in_=xk_f[:])
                for c in range(32):
                    rp = ps_a.tile([128, 512], F32, tag="pa")
                    nc.tensor.matmul(rp[0:96, 0:384], s4_sb[:, c, :], xko,
                                     start=True, stop=True)
                    n_t = st1.tile([96, 384], F32, tag="nt")
                    nc.vector.tensor_scalar(out=n_t, in0=rp[0:96, 0:384],
                                            scalar1=C_ROUND, scalar2=C_ROUND,
                                            op0=OP.add, op1=OP.subtract)
                    nf = st1.tile([96, 384], F32, tag="nf")
                    nc.vector.tensor_tensor(out=nf, in0=rp[0:96, 0:384], in1=n_t,
                                            op=OP.subtract)
                    nc.scalar.activation(out=trigc[c], in_=nf, func=AF.Sin,
                                         bias=0.0, scale=TWO_PI)
            tc.no_sync_barrier()
            if dbg:
                dbgt = sing.tile([96, 384], F32, tag="dbgt")
                nc.vector.tensor_copy(dbgt, trigc[0])
                nc.sync.dma_start(out=dbg_trig[:], in_=dbgt)
            W = load_weights()

            # ---------- initial features ----------
            f_full = fpool.tile([128, 768], BF16, tag="ffull")
            for half in range(2):
                p = ps_a.tile([128, 512], F32, tag="pa")
                nc.tensor.matmul(p[:, 0:384], W['enc'],
                                 W['feats'][:, half * 384:(half + 1) * 384],
                                 start=True, stop=True)
                nc.scalar.activation(out=f_full[:, half * 384:(half + 1) * 384],
                                     in_=p[:, 0:384], func=AF.Identity,
                                     bias=W['encb'], scale=1.0)
            fb = fpool.tile([128, 384], BF16, tag="fb")
            p = ps_a.tile([128, 512], F32, tag="pa")
            nc.tensor.matmul(p[:, 0:384], W['enc'], W['featsb'], start=True, stop=True)
            nc.scalar.activation(out=fb, in_=p[:, 0:384], func=AF.Identity,
                                 bias=W['encb'], scale=1.0)
            fq = fpool.tile([128, 96], BF16, tag="fq")
            p = ps_a.tile([128, 512], F32, tag="pa")
            nc.tensor.matmul(p[:, 0:96], W['enc'], W['featsq'], start=True, stop=True)
            nc.scalar.activation(out=fq, in_=p[:, 0:96], func=AF.Identity,
                                 bias=W['encb'], scale=1.0)

            pid = nc.scalar.partition_id()

            def affine_evict(src_ap, sc, b2, shape, dt=BF16, tag="aff", pool=None):
                t = (pool or fpool).tile(shape, dt, tag=tag)
                nc.scalar.activation(out=t, in_=src_ap, func=AF.Identity,
                                     bias=b2, scale=sc)
                return t

            # ---------- transformer blocks ----------
            for i in range(NB):
                # block consts
                pa = ps_a.tile([128, 512], F32, tag="pa")
                nc.tensor.matmul(pa[:, 0:384], W['nWk'][i], fb, start=True, stop=False)
                nc.tensor.matmul(pa[:, 0:384], W['nPd4'][i], W['xkb'], start=False, stop=True)
                EK = blk.tile([128, 384], BF16, tag="EK")
                nc.scalar.copy(EK, pa[:, 0:384])

                pb = ps_b.tile([128, 512], F32, tag="pb")
                nc.tensor.matmul(pb[:, 0:384], W['Wv'][i], fb, start=True, stop=False)
                nc.tensor.matmul(pb[:, 0:384], W['nPd4'][i], W['xkb'], start=False, stop=True)
                VK = blk.tile([128, 384], BF16, tag="VK")
                nc.scalar.copy(VK, pb[:, 0:384])

                pa = ps_a.tile([128, 512], F32, tag="pa")
                nc.tensor.matmul(pa[:, 0:96], W['G1Q'][i], fq, start=True, stop=False)
                nc.tensor.matmul(pa[:, 0:96], W['G1Pd4'][i], W['xqb'], start=False, stop=False)
                nc.tensor.matmul(pa[:, 0:96], W['c1'][i], ones96, start=False, stop=True)
                QB = blk.tile([128, 96], F32, tag="QB")
                nc.scalar.copy(QB, pa[:, 0:96])

                pb = ps_b.tile([128, 512], F32, tag="pb")
                nc.tensor.matmul(pb[:, 0:96], W['Pd4'][i], W['xqb'], start=True, stop=False)
                nc.tensor.matmul(pb[:, 0:96], W['bpe'][i], ones96, start=False, stop=True)
                QP = blk.tile([128, 96], F32, tag="QP")
                nc.scalar.copy(QP, pb[:, 0:96])

                S1 = blk.tile([128, 96], F32, tag="S1")
                S0 = blk.tile([128, 96], F32, tag="S0")

                # attention over 96 queries; exp per-query with accum (S0 free)
                for m in range(NQ):
                    tsl = trigc[m // 3][32 * (m % 3):32 * (m % 3) + 30, :]
                    hpre = ps_a.tile([128, 512], F32, tag="pa")
                    nc.tensor.matmul(
                        hpre[:, 0:384],
                        W['G1P'][i][32 * (m % 3):32 * (m % 3) + 30, :], tsl,
                        start=True, stop=False)
                    nc.tensor.matmul(hpre[:, 0:384], W['Wg1'][i], EK,
                                     start=False, stop=True)
                    h_t = hp.tile([128, 384], BF16, tag="h")
                    if m % 10 < RELU_ACT_FRAC10:
                        nc.scalar.activation(out=h_t, in_=hpre[:, 0:384],
                                             func=AF.Relu,
                                             bias=QB[:, m:m + 1], scale=1.0)
                    else:
                        nc.vector.tensor_scalar(
                            out=h_t, in0=hpre[:, 0:384],
                            scalar1=QB[:, m:m + 1], scalar2=0.0,
                            op0=OP.add, op1=OP.max)
                    lg = ps_g.tile([128, 512], F32, tag="lg")
                    nc.tensor.matmul(lg[:, 0:384], W['Wg2'][i], h_t,
                                     start=True, stop=True)
                    w_t = wp.tile([128, 384], BF16, tag="w")
                    nc.scalar.activation(out=w_t, in_=lg[:, 0:384], func=AF.Exp,
                                         bias=W['bg2'][i], scale=1.0,
                                         accum_out=S0[:, m:m + 1])
                    pos = ps_b.tile([128, 512], F32, tag="pb")
                    nc.tensor.matmul(
                        pos[:, 0:384],
                        W['Ps'][i][32 * (m % 3):32 * (m % 3) + 30, :], tsl,
                        start=True, stop=False)
                    nc.tensor.matmul(pos[:, 0:384], W['i128'], VK,
                                     start=False, stop=True)
                    nc.vector.scalar_tensor_tensor(
                        out=dumA.broadcast_to((128, 384)),
                        in0=pos[:, 0:384], scalar=QP[:, m:m + 1],
                        in1=w_t, op0=OP.add, op1=OP.mult,
                        accum_out=S1[:, m:m + 1])

                # block tail: o = S1/S0 + fq; stats; gather; BN
                if dbg and i == 0:
                    ek32 = wide.tile([128, 384], F32, tag="ek32")
                    nc.vector.tensor_copy(ek32, EK)
                    nc.sync.dma_start(out=dbg_EK[:], in_=ek32)
                    nc.sync.dma_start(out=dbg_QB[:], in_=QB)
                    nc.sync.dma_start(out=dbg_S0[:], in_=S0)
                    nc.sync.dma_start(out=dbg_S1[:], in_=S1)
                    fb32 = wide.tile([128, 384], F32, tag="fb32")
                    nc.vector.tensor_copy(fb32, fb)
                    nc.sync.dma_start(out=dbg_f0[:], in_=fb32)
                R = smalls.tile([128, 96], F32, tag="R")
                nc.vector.reciprocal(out=R, in_=S0)
                RES = smalls.tile([128, 96], F32, tag="RES")
                nc.vector.tensor_tensor(out=RES, in0=S1, in1=R, op=OP.mult)
                payload = blk.tile([128, 98], F32, tag="payload")
                nc.vector.tensor_tensor(out=payload[:, 0:96], in0=RES, in1=fq,
                                        op=OP.add)
                st = smalls.tile([128, 6], F32, tag="bnst")
                nc.vector.bn_stats(out=st, in_=payload[:, 0:96])
                mv = smalls.tile([128, 2], F32, tag="bnmv")
                nc.vector.bn_aggr(out=mv, in_=st)
                nc.vector.tensor_copy(payload[:, 96:97], mv[:, 0:1])
                msq = smalls.tile([128, 1], F32, tag="msq")
                nc.vector.tensor_tensor(out=msq, in0=mv[:, 0:1], in1=mv[:, 0:1],
                                        op=OP.mult)
                nc.vector.tensor_tensor(out=payload[:, 97:98], in0=mv[:, 1:2],
                                        in1=msq, op=OP.add)

                ag_in = dram.tile([128, 98], F32, tag=f"agin{i}")
                if variant == "spmd":
                    ag_out = dram.tile([8, 128, 98], F32, addr_space="Shared",
                                       tag=f"agout{i}")
                else:
                    ag_out = dram.tile([8, 128, 98], F32, tag=f"agout{i}")
                nc.gpsimd.dma_start(out=ag_in, in_=payload)
                if variant == "spmd":
                    nc.gpsimd.collective_compute(
                        "AllGather", OP.bypass, replica_groups=RG,
                        ins=[ag_in[:].opt()], outs=[ag_out[:].opt()])
                else:
                    for cc_ in range(8):
                        nc.sync.dma_start(out=ag_out[cc_], in_=payload)

                ago_ap = ag_out[:]
                o_full = wide.tile([128, 768], F32, tag="ofull")
                src = bass.AP(tensor=ago_ap.tensor, offset=ago_ap.offset,
                              ap=[[98, 128], [128 * 98, 8], [1, 96]])
                nc.gpsimd.dma_start(
                    out=o_full.rearrange("p (c k) -> p c k", c=8), in_=src)
                st8 = smalls.tile([128, 2, 8], F32, tag="st8")
                for jst in range(2):
                    src = bass.AP(tensor=ago_ap.tensor,
                                  offset=ago_ap.offset + 96 + jst,
                                  ap=[[98, 128], [128 * 98, 8]])
                    nc.sync.dma_start(out=st8[:, jst, :], in_=src)

                mg = smalls.tile([128, 1], F32, tag="mg")
                nc.vector.tensor_reduce(out=mg, in_=st8[:, 0, :],
                                        axis=mybir.AxisListType.X, op=OP.add)
                nc.vector.tensor_scalar(out=mg, in0=mg, scalar1=0.125,
                                        scalar2=None, op0=OP.mult)
                e2g = smalls.tile([128, 1], F32, tag="e2g")
                nc.vector.tensor_reduce(out=e2g, in_=st8[:, 1, :],
                                        axis=mybir.AxisListType.X, op=OP.add)
                nc.vector.tensor_scalar(out=e2g, in0=e2g, scalar1=0.125,
                                        scalar2=None, op0=OP.mult)
                var = smalls.tile([128, 1], F32, tag="var")
                nc.vector.tensor_tensor(out=var, in0=mg, in1=mg, op=OP.mult)
                nc.vector.tensor_tensor(out=var, in0=e2g, in1=var, op=OP.subtract)
                lnv = smalls.tile([128, 1], F32, tag="lnv")
                nc.scalar.activation(out=lnv, in_=var, func=AF.Ln, bias=eps_t,
                                     scale=1.0)
                rs = smalls.tile([128, 1], F32, tag="rs")
                nc.scalar.activation(out=rs, in_=lnv, func=AF.Exp, bias=0.0,
                                     scale=-0.5)
                sc = smalls.tile([128, 1], F32, tag="sc")
                nc.vector.tensor_tensor(out=sc, in0=W['gam'][i], in1=rs, op=OP.mult)
                b2 = smalls.tile([128, 1], F32, tag="b2")
                nc.vector.tensor_scalar(out=b2, in0=mg, scalar1=sc, scalar2=None,
                                        op0=OP.mult)
                nc.vector.tensor_tensor(out=b2, in0=W['bet'][i], in1=b2, op=OP.subtract)

                f_full = affine_evict(o_full[:], sc, b2, [128, 768], tag="ffull")
                fq = fpool.tile([128, 96], BF16, tag="fq")
                nc.vector.tensor_scalar(out=fq, in0=payload[:, 0:96], scalar1=sc,
                                        scalar2=b2, op0=OP.mult, op1=OP.add)
                fb = fpool.tile([128, 384], BF16, tag="fb")
                with tc.If(pid < 4) as cmp:
                    nc.scalar.activation(out=fb, in_=o_full[:, 0:384],
                                         func=AF.Identity, bias=b2, scale=sc)
                with cmp.Else():
                    nc.scalar.activation(out=fb, in_=o_full[:, 384:768],
                                         func=AF.Identity, bias=b2, scale=sc)

                # ---------- MLP ----------
                if i > 0:
                    j = i - 1

                    def mlp_layer(lw, bias_ap, xin, width, tag):
                        t = wide.tile([128, width], BF16, tag=tag)
                        for h0 in range(0, width, 384):
                            wdt = min(384, width - h0)
                            pp = ps_a.tile([128, 512], F32, tag="pa")
                            nc.tensor.matmul(pp[:, 0:wdt], lw,
                                             xin[:, h0:h0 + wdt],
                                             start=True, stop=True)
                            nc.scalar.activation(out=t[:, h0:h0 + wdt],
                                                 in_=pp[:, 0:wdt], func=AF.Relu,
                                                 bias=bias_ap, scale=1.0)
                        return t

                    y1f = mlp_layer(W['em1'][j], W['emb1'][j], f_full, 768, "y1f")
                    y2f = mlp_layer(W['em2'][j], W['emb2'][j], y1f, 768, "y2f")
                    o2f = wide.tile([128, 768], F32, tag="o2f")
                    nc.vector.tensor_tensor(out=o2f, in0=f_full, in1=y2f, op=OP.add)
                    y1b = mlp_layer(W['em1'][j], W['emb1'][j], fb, 384, "y1b")
                    y2b = mlp_layer(W['em2'][j], W['emb2'][j], y1b, 384, "y2b")
                    o2b = wide.tile([128, 384], F32, tag="o2b")
                    nc.vector.tensor_tensor(out=o2b, in0=fb, in1=y2b, op=OP.add)
                    y1q = mlp_layer(W['em1'][j], W['emb1'][j], fq, 96, "y1q")
                    y2q = mlp_layer(W['em2'][j], W['emb2'][j], y1q, 96, "y2q")
                    o2q = wide.tile([128, 96], F32, tag="o2q")
                    nc.vector.tensor_tensor(out=o2q, in0=fq, in1=y2q, op=OP.add)

                    st2 = smalls.tile([128, 2, 6], F32, tag="st2")
                    nc.vector.bn_stats(out=st2[:, 0, :], in_=o2f[:, 0:384])
                    nc.vector.bn_stats(out=st2[:, 1, :], in_=o2f[:, 384:768])
                    mv2 = smalls.tile([128, 2], F32, tag="mv2")
                    nc.vector.bn_aggr(out=mv2, in_=st2)
                    lnv2 = smalls.tile([128, 1], F32, tag="lnv")
                    nc.scalar.activation(out=lnv2, in_=mv2[:, 1:2], func=AF.Ln,
                                         bias=eps_t, scale=1.0)
                    rs2 = smalls.tile([128, 1], F32, tag="rs")
                    nc.scalar.activation(out=rs2, in_=lnv2, func=AF.Exp, bias=0.0,
                                         scale=-0.5)
                    sc2 = smalls.tile([128, 1], F32, tag="sc")
                    nc.vector.tensor_tensor(out=sc2, in0=W['emg'][j], in1=rs2,
                                            op=OP.mult)
                    b22 = smalls.tile([128, 1], F32, tag="b2")
                    nc.vector.tensor_scalar(out=b22, in0=mv2[:, 0:1], scalar1=sc2,
                                            scalar2=None, op0=OP.mult)
                    nc.vector.tensor_tensor(out=b22, in0=W['embe'][j], in1=b22,
                                            op=OP.subtract)
                    f_full = affine_evict(o2f[:], sc2, b22, [128, 768], tag="ffull")
                    fb = fpool.tile([128, 384], BF16, tag="fb")
                    nc.vector.tensor_scalar(out=fb, in0=o2b, scalar1=sc2,
                                            scalar2=b22, op0=OP.mult, op1=OP.add)
                    fq = fpool.tile([128, 96], BF16, tag="fq")
                    nc.vector.tensor_scalar(out=fq, in0=o2q, scalar1=sc2,
                                            scalar2=b22, op0=OP.mult, op1=OP.add)

            # ---------- final FC + max ----------
            for bb in range(2):
                fbb = f_full[:, bb * 384:(bb + 1) * 384]
                e1 = []
                for h in range(2):
                    pp = ps_a.tile([128, 512], F32, tag="pa")
                    nc.tensor.matmul(pp[:, 0:384], W['f1'][h], fbb, start=True,
                                     stop=True)
                    e1t = wide.tile([128, 384], BF16, tag=f"e1{h}")
                    nc.scalar.activation(out=e1t, in_=pp[:, 0:384], func=AF.Relu,
                                         bias=W['f1b'][h], scale=1.0)
                    e1.append(e1t)
                for h in range(2):
                    pp = ps_b.tile([128, 512], F32, tag="pb")
                    nc.tensor.matmul(pp[:, 0:384], W['f2'][h][0], e1[0], start=True,
                                     stop=False)
                    nc.tensor.matmul(pp[:, 0:384], W['f2'][h][1], e1[1], start=False,
                                     stop=True)
                    mx = smalls.tile([128, 1], F32, tag="mx")
                    nc.vector.tensor_reduce(out=mx, in_=pp[:, 0:384],
                                            axis=mybir.AxisListType.X, op=OP.max)
                    ot = smalls.tile([128, 1], F32, tag="ot")
                    nc.vector.tensor_scalar(out=ot, in0=mx, scalar1=W['f2b'][h],
                                            scalar2=None, op0=OP.add)
                    nc.sync.dma_start(
                        out=out_d[bb:bb + 1, h * 128:(h + 1) * 128], in_=ot)

    nc.compile()
    _CACHE[variant] = nc
    return nc


def _prep_inputs(inputs):
    """Host-side constant relayout + per-core slicing. Returns in_maps list."""
    xyz = _f32(inputs["xyz"])          # [2, 384, 3]
    feats = _f32(inputs["feats"])      # [2, 384, 1]

    Wq, Wk, Wv = inputs["tb_Wq"], inputs["tb_Wk"], inputs["tb_Wv"]
    Wg1, bg1 = inputs["tb_Wg1"], inputs["tb_bg1"]
    Wg2, bg2 = inputs["tb_Wg2"], inputs["tb_bg2"]
    Wpe, bpe = inputs["tb_Wpe"], inputs["tb_bpe"]

    L_G1P = np.zeros((NB, 30, 128), np.float32)
    L_Ps = np.zeros((NB, 30, 128), np.float32)
    L_nWk = np.zeros((NB, 128, 128), np.float32)
    L_Wv = np.zeros((NB, 128, 128), np.float32)
    L_nPd4 = np.zeros((NB, 3, 128), np.float32)
    L_G1Q = np.zeros((NB, 128, 128), np.float32)
    L_G1Pd4 = np.zeros((NB, 3, 128), np.float32)
    L_Pd4 = np.zeros((NB, 3, 128), np.float32)
    L_Wg1 = np.zeros((NB, 128, 128), np.float32)
    L_Wg2 = np.zeros((NB, 128, 128), np.float32)
    row_c1 = np.zeros((NB, 1, 128), np.float32)
    row_bpe = np.zeros((NB, 1, 128), np.float32)
    for i in range(NB):
        Ws, Wd = _wpe_split(_f32(Wpe[i]))
        g1 = _f32(Wg1[i])
        L_G1P[i] = (g1 @ Ws).T
        L_Ps[i] = Ws.T
        L_nWk[i] = (-_f32(Wk[i])).T
        L_Wv[i] = _f32(Wv[i]).T
        L_nPd4[i] = (-4.0 * Wd).T
        L_G1Q[i] = (g1 @ _f32(Wq[i])).T
        L_G1Pd4[i] = (4.0 * (g1 @ Wd)).T
        L_Pd4[i] = (4.0 * Wd).T
        L_Wg1[i] = g1.T
        L_Wg2[i] = _f32(Wg2[i]).T
        row_c1[i, 0] = g1 @ _f32(bpe[i]) + _f32(bg1[i])
        row_bpe[i, 0] = _f32(bpe[i])

    W2 = _f32(inputs["fcf_W2"])
    L_f2 = np.zeros((2, 2, 128, 128), np.float32)
    for h in range(2):
        for k in range(2):
            L_f2[h, k] = W2.T[k * 128:(k + 1) * 128, h * 128:(h + 1) * 128]

    com = {
        "feats_row": _bf(feats.reshape(1, 768)),
        "i128": _bf(np.eye(128, dtype=np.float32)),
        "L_G1P": _bf(L_G1P), "L_Wg1": _bf(L_Wg1), "L_Wg2": _bf(L_Wg2),
        "L_Ps": _bf(L_Ps), "L_nWk": _bf(L_nWk), "L_Wv": _bf(L_Wv),
        "L_nPd4": _bf(L_nPd4), "L_G1Q": _bf(L_G1Q), "L_G1Pd4": _bf(L_G1Pd4),
        "L_Pd4": _bf(L_Pd4), "row_c1": _bf(row_c1), "row_bpe": _bf(row_bpe),
        "bg2_v": _f32(bg2).reshape(NB, 128, 1),
        "gamma_v": _f32(inputs["tb_gamma"]).reshape(NB, 128, 1),
        "beta_v": _f32(inputs["tb_beta"]).reshape(NB, 128, 1),
        "L_enc": _bf(_f32(inputs["enc_W"])[:, 0:1].T),
        "encb_v": _f32(inputs["enc_b"]).reshape(128, 1),
        "L_em1": _bf(np.stack([_f32(inputs["em_W1"][j]).T for j in range(NF)])),
        "L_em2": _bf(np.stack([_f32(inputs["em_W2"][j]).T for j in range(NF)])),
        "em_b1_v": _f32(inputs["em_b1"]).reshape(NF, 128, 1),
        "em_b2_v": _f32(inputs["em_b2"]).reshape(NF, 128, 1),
        "em_g_v": _f32(inputs["em_gamma"]).reshape(NF, 128, 1),
        "em_be_v": _f32(inputs["em_beta"]).reshape(NF, 128, 1),
        "L_f1": _bf(_f32(inputs["fcf_W1"]).T.reshape(128, 2, 128).transpose(1, 0, 2)),
        "f1_b": _f32(inputs["fcf_b1"]).reshape(2, 128, 1),
        "L_f2": _bf(L_f2),
        "f2_b": _f32(inputs["fcf_b2"]).reshape(2, 128, 1),
    }

    in_maps = []
    for c in range(8):
        b, qo = c // 4, (c % 4) * 96
        xk = xyz[b].T                      # [3, 384]
        S4 = np.zeros((4, 32, 96), np.float32)
        for cch in range(32):
            for qr in range(3):
                qg = qo + 3 * cch + qr
                for j in range(3):
                    for t in range(10):
                        col = 32 * qr + 10 * j + t
                        s = np.float32(4.0 * FREQS[t % 5] / TWO_PI)
                        off = np.float32(0.25 if t >= 5 else 0.0)
                        S4[j, cch, col] = s
                        S4[3, cch, col] = off - s * np.float32(xyz[b, qg, j])
        m = dict(com)
        m["xk_f"] = _f32(xk)
        m["s4_f"] = _f32(S4.reshape(4, 32 * 96))
        m["xk_b"] = _bf(xk)
        m["xq_b"] = _bf(xk[:, qo:qo + 96])
        m["feats_b"] = _bf(feats[b].reshape(1, 384))
        m["feats_q"] = _bf(feats[b, qo:qo + 96].reshape(1, 96))
        in_maps.append(m)
    return in_maps


def kernel(**inputs):
    from concourse.bass_utils import run_bass_kernel_spmd

    nc = _build()
    in_maps = _prep_inputs(inputs)
    res = run_bass_kernel_spmd(nc, in_maps, list(range(8)))
    return np.asarray(res.results[0]["out"], np.float32)


if __name__ == "__main__":
    rng = np.random.RandomState(0)
    fake = {
        "xyz": rng.randn(2, 384, 3).astype(np.float32),
        "feats": rng.randn(2, 384, 1).astype(np.float32),
    }
    print("smoke build only")



# revision 10
# speedup vs baseline: 1.0005x; 1.0005x over previous
"""NePuEncoder Bass/Tile kernel for 8 Trainium2 NeuronCores.

Sharding: query-parallel. Core c handles batch b=c//4, queries qo=(c%4)*96 ..
qo+96 of that batch. Channel-major layout [128 chan, keys] throughout.

Per-channel softmax attention fully fused in SBUF/PSUM:
  - pairwise trig features via range-reduced Sin (|arg|<=pi), computed once
    (32 chunks of 3 queries) and reused across all 3 transformer blocks
  - per-query: 5 bf16 matmuls (trig->hpre, +Wg1@EK accum, logits, trig->pos,
    +I@VK accum), ACT relu (per-query bias) + batched ACT exp, DVE
    scalar_tensor_tensor (pos+QP)*w accumulate (S1) and tensor_scalar
    accumulate (S0 = sum w)
  - one 8-rank AllGather per block carries o=res+f plus per-chunk BN stats;
    BatchNorm applied replicated. MLP blocks and the final FC are computed
    redundantly per core (cheap); core 0's output is returned.

All constant tensors are packed host-side into 5 DRAM tensors loaded with 5
DMAs at program start (HWDGE issue overhead dominates small loads).
"""
import sys

sys.path.insert(0, "/opt/trn_rl_repo")

import numpy as np
import ml_dtypes

B, N, D, DS, LAT, FD, NF = 2, 384, 128, 3, 256, 1, 2
NB = NF + 1
NQ = 96                  # queries per core
NCH = 32                 # trig chunks (3 queries each)
RELU_ACT_FRAC10 = 3      # m%10 < this -> relu on ACT, else DVE
FREQS = np.linspace(1.0, 32.0, 5).astype(np.float64)
EPS = 1e-5
TWO_PI = float(2 * np.pi)
C_ROUND = float(3 << 22)  # 2^23 + 2^22: fp32 round-to-nearest-even trick

BF = ml_dtypes.bfloat16

# --- wpack column offsets (bf16 [128, WCOLS]) ---
W_WG1, W_WG2, W_NWK, W_WV, W_G1Q = 0, 384, 768, 1152, 1536
W_G1P, W_PS, W_I128 = 1920, 2304, 2688
W_EM1, W_EM2, W_F1, W_F2 = 2816, 3072, 3328, 3584
WCOLS = 4096
# --- vpack column offsets (f32 [128, VCOLS]) ---
V_BG2, V_GAM, V_BET, V_ENCB = 0, 3, 6, 9
V_EMB1, V_EMB2, V_EMG, V_EMBE, V_F1B, V_F2B = 10, 12, 14, 16, 18, 20
VCOLS = 22
# --- rpack column offsets (bf16 [3, RCOLS]) ---
R_XKB, R_NPD4, R_G1PD4, R_PD4, R_XQB = 0, 384, 768, 1152, 1536
RCOLS = 1632
# --- qpack column offsets (bf16 [1, QCOLS]) ---
Q_FEATS, Q_ENC, Q_C1, Q_BPE, Q_FB, Q_FQ = 0, 768, 896, 1280, 1664, 2048
QCOLS = 2144
# --- fpack column offsets (f32 [4, FCOLS]) ---
F_S4, F_XKO = 0, NCH * 96
FCOLS = NCH * 96 + 384


def _bf(x):
    return np.ascontiguousarray(np.asarray(x, np.float32).astype(BF))


def _f32(x):
    return np.ascontiguousarray(np.asarray(x, np.float32))


def _wpe_split(Wpe):
    """W_s [128,30] trig cols (sin-sign absorbed for npd = xk - xq), W_d [128,3]."""
    Ws = np.zeros((D, 30), np.float32)
    for j in range(3):
        for t in range(10):
            r = 10 * j + t
            if t < 5:
                Ws[:, r] = -Wpe[:, 3 + 6 * t + j]
            else:
                Ws[:, r] = Wpe[:, 3 + 6 * (t - 5) + 3 + j]
    return Ws, Wpe[:, 0:3].astype(np.float32)


def _rep3(M30):
    """[30,128] -> [128,128] with copies at partition offsets 0/32/64."""
    out = np.zeros((128, 128), np.float32)
    for s in range(3):
        out[32 * s:32 * s + 30, :] = M30
    return out


_CACHE = {}


def _build(variant="spmd"):
    if variant in _CACHE:
        return _CACHE[variant]

    import concourse.bacc as bacc
    import concourse.bass as bass
    import concourse.tile as tile
    from concourse import mybir

    F32, BF16 = mybir.dt.float32, mybir.dt.bfloat16
    AF = mybir.ActivationFunctionType
    OP = mybir.AluOpType

    nc = bacc.Bacc(None, target_bir_lowering=False,
                   num_devices=(8 if variant == "spmd" else 1))

    wpack_d = nc.dram_tensor("wpack", [128, WCOLS], BF16, kind="ExternalInput")
    vpack_d = nc.dram_tensor("vpack", [128, VCOLS], F32, kind="ExternalInput")
    rpack_d = nc.dram_tensor("rpack", [3, RCOLS], BF16, kind="ExternalInput")
    qpack_d = nc.dram_tensor("qpack", [1, QCOLS], BF16, kind="ExternalInput")
    fpack_d = nc.dram_tensor("fpack", [4, FCOLS], F32, kind="ExternalInput")

    out_d = nc.dram_tensor("out", [2, 256], F32, kind="ExternalOutput")
    RG = [[0, 1, 2, 3, 4, 5, 6, 7]]

    with tile.TileContext(nc) as tc:
        with (
            tc.tile_pool(name="sing", bufs=1) as sing,
            tc.tile_pool(name="fpool", bufs=2) as fpool,
            tc.tile_pool(name="blk", bufs=2) as blk,
            tc.tile_pool(name="hp", bufs=4) as hp,
            tc.tile_pool(name="wp", bufs=4) as wp,
            tc.tile_pool(name="wide", bufs=2) as wide,
            tc.tile_pool(name="smalls", bufs=4) as smalls,
            tc.tile_pool(name="st1", bufs=3) as st1,
            tc.tile_pool(name="ps_a", bufs=4, space="PSUM") as ps_a,
            tc.tile_pool(name="ps_b", bufs=2, space="PSUM") as ps_b,
            tc.tile_pool(name="ps_g", bufs=2, space="PSUM") as ps_g,
            tc.tile_pool(name="dram", bufs=1, space="DRAM") as dram,
        ):
            # ---------- packed constant loads: 5 DMAs, issued first ----------
            wpack = sing.tile([128, WCOLS], BF16, tag="wpack", name="wpack")
            nc.sync.dma_start(out=wpack, in_=wpack_d[:])
            vpack = sing.tile([128, VCOLS], F32, tag="vpack", name="vpack")
            nc.scalar.dma_start(out=vpack, in_=vpack_d[:])
            rpack = sing.tile([3, RCOLS], BF16, tag="rpack", name="rpack")
            nc.scalar.dma_start(out=rpack, in_=rpack_d[:])
            qpack = sing.tile([1, QCOLS], BF16, tag="qpack", name="qpack")
            nc.sync.dma_start(out=qpack, in_=qpack_d[:])
            fpack = sing.tile([4, FCOLS], F32, tag="fpack", name="fpack")
            nc.sync.dma_start(out=fpack, in_=fpack_d[:])

            def wsl(off, i=0):
                return wpack[:, off + 128 * i: off + 128 * (i + 1)]

            W = {
                "Wg1": [wsl(W_WG1, i) for i in range(NB)],
                "Wg2": [wsl(W_WG2, i) for i in range(NB)],
                "nWk": [wsl(W_NWK, i) for i in range(NB)],
                "Wv": [wsl(W_WV, i) for i in range(NB)],
                "G1Q": [wsl(W_G1Q, i) for i in range(NB)],
                "i128": wsl(W_I128),
                "em1": [wsl(W_EM1, j) for j in range(NF)],
                "em2": [wsl(W_EM2, j) for j in range(NF)],
                "f1": [wsl(W_F1, h) for h in range(2)],
                "f2": [[wsl(W_F2, 2 * h + k) for k in range(2)] for h in range(2)],
                "bg2": [vpack[:, V_BG2 + i: V_BG2 + i + 1] for i in range(NB)],
                "gam": [vpack[:, V_GAM + i: V_GAM + i + 1] for i in range(NB)],
                "bet": [vpack[:, V_BET + i: V_BET + i + 1] for i in range(NB)],
                "encb": vpack[:, V_ENCB: V_ENCB + 1],
                "emb1": [vpack[:, V_EMB1 + j: V_EMB1 + j + 1] for j in range(NF)],
                "emb2": [vpack[:, V_EMB2 + j: V_EMB2 + j + 1] for j in range(NF)],
                "emg": [vpack[:, V_EMG + j: V_EMG + j + 1] for j in range(NF)],
                "embe": [vpack[:, V_EMBE + j: V_EMBE + j + 1] for j in range(NF)],
                "f1b": [vpack[:, V_F1B + h: V_F1B + h + 1] for h in range(2)],
                "f2b": [vpack[:, V_F2B + h: V_F2B + h + 1] for h in range(2)],
                "xkb": rpack[:, R_XKB: R_XKB + 384],
                "xqb": rpack[:, R_XQB: R_XQB + 96],
                "nPd4": [rpack[:, R_NPD4 + 128 * i: R_NPD4 + 128 * (i + 1)] for i in range(NB)],
                "G1Pd4": [rpack[:, R_G1PD4 + 128 * i: R_G1PD4 + 128 * (i + 1)] for i in range(NB)],
                "Pd4": [rpack[:, R_PD4 + 128 * i: R_PD4 + 128 * (i + 1)] for i in range(NB)],
                "feats": qpack[:, Q_FEATS: Q_FEATS + 768],
                "enc": qpack[:, Q_ENC: Q_ENC + 128],
                "c1": [qpack[:, Q_C1 + 128 * i: Q_C1 + 128 * (i + 1)] for i in range(NB)],
                "bpe": [qpack[:, Q_BPE + 128 * i: Q_BPE + 128 * (i + 1)] for i in range(NB)],
                "featsb": qpack[:, Q_FB: Q_FB + 384],
                "featsq": qpack[:, Q_FQ: Q_FQ + 96],
            }

            ones96 = sing.tile([1, 96], BF16, tag="ones96")
            nc.vector.memset(ones96, 1.0)
            eps_t = sing.tile([128, 1], F32, tag="epst")
            nc.vector.memset(eps_t, EPS)
            dumA = sing.tile([128, 1], BF16, tag="dumA")

            trigc = [sing.tile([96, 384], BF16, tag=f"trig{c}", name=f"trig{c}")
                     for c in range(NCH)]

            # ---------- stage 1: trig features via S-matmul + range-reduced Sin ----
            # r[row,k] = s*(xk[j,k]-xq[q,j]) + off  computed as fp32 matmul
            # S4 @ [xk;ones];  n = (r+C)-C (round);  -frac = (n-C)-r;
            # trig = sin(-2pi * -frac) -> bf16, query q_rel at partitions 32*q_rel.
            xko = fpack[:, F_XKO: F_XKO + 384]
            for c in range(NCH):
                rp = ps_a.tile([128, 512], F32, tag="pa")
                nc.tensor.matmul(rp[0:96, 0:384],
                                 fpack[:, F_S4 + 96 * c: F_S4 + 96 * c + 96],
                                 xko, start=True, stop=True)
                n_t = st1.tile([96, 384], F32, tag="nt")
                nc.vector.tensor_scalar(out=n_t, in0=rp[0:96, 0:384],
                                        scalar1=C_ROUND, scalar2=C_ROUND,
                                        op0=OP.add, op1=OP.subtract)
                nf = st1.tile([96, 384], F32, tag="nf")
                nc.vector.tensor_tensor(out=nf, in0=rp[0:96, 0:384], in1=n_t,
                                        op=OP.subtract)
                nc.scalar.activation(out=trigc[c], in_=nf, func=AF.Sin,
                                     bias=0.0, scale=TWO_PI)

            # ---------- initial features ----------
            f_full = fpool.tile([128, 768], BF16, tag="ffull")
            for half in range(2):
                p = ps_a.tile([128, 512], F32, tag="pa")
                nc.tensor.matmul(p[:, 0:384], W['enc'],
                                 W['feats'][:, half * 384:(half + 1) * 384],
                                 start=True, stop=True)
                nc.scalar.activation(out=f_full[:, half * 384:(half + 1) * 384],
                                     in_=p[:, 0:384], func=AF.Identity,
                                     bias=W['encb'], scale=1.0)
            fb = fpool.tile([128, 384], BF16, tag="fb")
            p = ps_a.tile([128, 512], F32, tag="pa")
            nc.tensor.matmul(p[:, 0:384], W['enc'], W['featsb'], start=True, stop=True)
            nc.scalar.activation(out=fb, in_=p[:, 0:384], func=AF.Identity,
                                 bias=W['encb'], scale=1.0)
            fq = fpool.tile([128, 96], BF16, tag="fq")
            p = ps_a.tile([128, 512], F32, tag="pa")
            nc.tensor.matmul(p[:, 0:96], W['enc'], W['featsq'], start=True, stop=True)
            nc.scalar.activation(out=fq, in_=p[:, 0:96], func=AF.Identity,
                                 bias=W['encb'], scale=1.0)

            pid = nc.scalar.partition_id()

            def affine_evict(src_ap, sc, b2, shape, dt=BF16, tag="aff", pool=None):
                t = (pool or fpool).tile(shape, dt, tag=tag)
                nc.scalar.activation(out=t, in_=src_ap, func=AF.Identity,
                                     bias=b2, scale=sc)
                return t

            # ---------- transformer blocks ----------
            for i in range(NB):
                # block consts
                pa = ps_a.tile([128, 512], F32, tag="pa")
                nc.tensor.matmul(pa[:, 0:384], W['nWk'][i], fb, start=True, stop=False)
                nc.tensor.matmul(pa[:, 0:384], W['nPd4'][i], W['xkb'], start=False, stop=True)
                EK = blk.tile([128, 384], BF16, tag="EK")
                nc.scalar.copy(EK, pa[:, 0:384])

                pb = ps_b.tile([128, 512], F32, tag="pb")
                nc.tensor.matmul(pb[:, 0:384], W['Wv'][i], fb, start=True, stop=False)
                nc.tensor.matmul(pb[:, 0:384], W['nPd4'][i], W['xkb'], start=False, stop=True)
                VK = blk.tile([128, 384], BF16, tag="VK")
                nc.scalar.copy(VK, pb[:, 0:384])

                pa = ps_a.tile([128, 512], F32, tag="pa")
                nc.tensor.matmul(pa[:, 0:96], W['G1Q'][i], fq, start=True, stop=False)
                nc.tensor.matmul(pa[:, 0:96], W['G1Pd4'][i], W['xqb'], start=False, stop=False)
                nc.tensor.matmul(pa[:, 0:96], W['c1'][i], ones96, start=False, stop=True)
                QB = blk.tile([128, 96], F32, tag="QB")
                nc.scalar.copy(QB, pa[:, 0:96])

                pb = ps_b.tile([128, 512], F32, tag="pb")
                nc.tensor.matmul(pb[:, 0:96], W['Pd4'][i], W['xqb'], start=True, stop=False)
                nc.tensor.matmul(pb[:, 0:96], W['bpe'][i], ones96, start=False, stop=True)
                QP = blk.tile([128, 96], F32, tag="QP")
                nc.scalar.copy(QP, pb[:, 0:96])

                S1 = blk.tile([128, 96], F32, tag="S1")
                S0 = blk.tile([128, 96], F32, tag="S0")

                # attention over 96 queries; exp per-query with accum (S0 free)
                for m in range(NQ):
                    s4o = 32 * (m % 3)
                    tsl = trigc[m // 3][s4o:s4o + 30, :]
                    hpre = ps_a.tile([128, 512], F32, tag="pa")
                    nc.tensor.matmul(
                        hpre[:, 0:384],
                        wpack[s4o:s4o + 30, W_G1P + 128 * i: W_G1P + 128 * (i + 1)],
                        tsl, start=True, stop=False)
                    nc.tensor.matmul(hpre[:, 0:384], W['Wg1'][i], EK,
                                     start=False, stop=True)
                    h_t = hp.tile([128, 384], BF16, tag="h")
                    if m % 10 < RELU_ACT_FRAC10:
                        nc.scalar.activation(out=h_t, in_=hpre[:, 0:384],
                                             func=AF.Relu,
                                             bias=QB[:, m:m + 1], scale=1.0)
                    else:
                        nc.vector.tensor_scalar(
                            out=h_t, in0=hpre[:, 0:384],
                            scalar1=QB[:, m:m + 1], scalar2=0.0,
                            op0=OP.add, op1=OP.max)
                    lg = ps_g.tile([128, 512], F32, tag="lg")
                    nc.tensor.matmul(lg[:, 0:384], W['Wg2'][i], h_t,
                                     start=True, stop=True)
                    w_t = wp.tile([128, 384], BF16, tag="w")
                    nc.scalar.activation(out=w_t, in_=lg[:, 0:384], func=AF.Exp,
                                         bias=W['bg2'][i], scale=1.0,
                                         accum_out=S0[:, m:m + 1])
                    pos = ps_b.tile([128, 512], F32, tag="pb")
                    nc.tensor.matmul(
                        pos[:, 0:384],
                        wpack[s4o:s4o + 30, W_PS + 128 * i: W_PS + 128 * (i + 1)],
                        tsl, start=True, stop=False)
                    nc.tensor.matmul(pos[:, 0:384], W['i128'], VK,
                                     start=False, stop=True)
                    nc.vector.scalar_tensor_tensor(
                        out=dumA.broadcast_to((128, 384)),
                        in0=pos[:, 0:384], scalar=QP[:, m:m + 1],
                        in1=w_t, op0=OP.add, op1=OP.mult,
                        accum_out=S1[:, m:m + 1])

                # block tail: o = S1/S0 + fq; stats; gather; BN
                R = smalls.tile([128, 96], F32, tag="R")
                nc.vector.reciprocal(out=R, in_=S0)
                RES = smalls.tile([128, 96], F32, tag="RES")
                nc.vector.tensor_tensor(out=RES, in0=S1, in1=R, op=OP.mult)
                payload = blk.tile([128, 98], F32, tag="payload")
                nc.vector.tensor_tensor(out=payload[:, 0:96], in0=RES, in1=fq,
                                        op=OP.add)
                st = smalls.tile([128, 6], F32, tag="bnst")
                nc.vector.bn_stats(out=st, in_=payload[:, 0:96])
                mv = smalls.tile([128, 2], F32, tag="bnmv")
                nc.vector.bn_aggr(out=mv, in_=st)
                nc.vector.tensor_copy(payload[:, 96:97], mv[:, 0:1])
                msq = smalls.tile([128, 1], F32, tag="msq")
                nc.vector.tensor_tensor(out=msq, in0=mv[:, 0:1], in1=mv[:, 0:1],
                                        op=OP.mult)
                nc.vector.tensor_tensor(out=payload[:, 97:98], in0=mv[:, 1:2],
                                        in1=msq, op=OP.add)

                ag_in = dram.tile([128, 98], F32, tag=f"agin{i}")
                if variant == "spmd":
                    ag_out = dram.tile([8, 128, 98], F32, addr_space="Shared",
                                       tag=f"agout{i}")
                else:
                    ag_out = dram.tile([8, 128, 98], F32, tag=f"agout{i}")
                nc.gpsimd.dma_start(out=ag_in, in_=payload)
                if variant == "spmd":
                    nc.gpsimd.collective_compute(
                        "AllGather", OP.bypass, replica_groups=RG,
                        ins=[ag_in[:].opt()], outs=[ag_out[:].opt()])
                else:
                    for cc_ in range(8):
                        eng = nc.sync if cc_ % 2 == 0 else nc.scalar
                        eng.dma_start(out=ag_out[cc_], in_=payload)

                ago_ap = ag_out[:]
                o_full = wide.tile([128, 768], F32, tag="ofull")
                for hh in range(2):
                    src = bass.AP(tensor=ago_ap.tensor,
                                  offset=ago_ap.offset + 128 * 98 * 4 * hh,
                                  ap=[[98, 128], [128 * 98, 4], [1, 96]])
                    eng = nc.sync if hh == 0 else nc.scalar
                    eng.dma_start(
                        out=o_full[:, 384 * hh:384 * (hh + 1)].rearrange(
                            "p (c k) -> p c k", c=4),
                        in_=src)
                st8 = smalls.tile([128, 2, 8], F32, tag="st8")
                for jst in range(2):
                    src = bass.AP(tensor=ago_ap.tensor,
                                  offset=ago_ap.offset + 96 + jst,
                                  ap=[[98, 128], [128 * 98, 8]])
                    nc.gpsimd.dma_start(out=st8[:, jst, :], in_=src)

                mg = smalls.tile([128, 1], F32, tag="mg")
                nc.vector.tensor_reduce(out=mg, in_=st8[:, 0, :],
                                        axis=mybir.AxisListType.X, op=OP.add)
                nc.vector.tensor_scalar(out=mg, in0=mg, scalar1=0.125,
                                        scalar2=None, op0=OP.mult)
                e2g = smalls.tile([128, 1], F32, tag="e2g")
                nc.vector.tensor_reduce(out=e2g, in_=st8[:, 1, :],
                                        axis=mybir.AxisListType.X, op=OP.add)
                nc.vector.tensor_scalar(out=e2g, in0=e2g, scalar1=0.125,
                                        scalar2=None, op0=OP.mult)
                var = smalls.tile([128, 1], F32, tag="var")
                nc.vector.tensor_tensor(out=var, in0=mg, in1=mg, op=OP.mult)
                nc.vector.tensor_tensor(out=var, in0=e2g, in1=var, op=OP.subtract)
                lnv = smalls.tile([128, 1], F32, tag="lnv")
                nc.scalar.activation(out=lnv, in_=var, func=AF.Ln, bias=eps_t,
                                     scale=1.0)
                rs = smalls.tile([128, 1], F32, tag="rs")
                nc.scalar.activation(out=rs, in_=lnv, func=AF.Exp, bias=0.0,
                                     scale=-0.5)
                sc = smalls.tile([128, 1], F32, tag="sc")
                nc.vector.tensor_tensor(out=sc, in0=W['gam'][i], in1=rs, op=OP.mult)
                b2 = smalls.tile([128, 1], F32, tag="b2")
                nc.vector.tensor_scalar(out=b2, in0=mg, scalar1=sc, scalar2=None,
                                        op0=OP.mult)
                nc.vector.tensor_tensor(out=b2, in0=W['bet'][i], in1=b2, op=OP.subtract)

                f_full = affine_evict(o_full[:], sc, b2, [128, 768], tag="ffull")
                fq = fpool.tile([128, 96], BF16, tag="fq")
                nc.vector.tensor_scalar(out=fq, in0=payload[:, 0:96], scalar1=sc,
                                        scalar2=b2, op0=OP.mult, op1=OP.add)
                fb = fpool.tile([128, 384], BF16, tag="fb")
                with tc.If(pid < 4) as cmp:
                    nc.scalar.activation(out=fb, in_=o_full[:, 0:384],
                                         func=AF.Identity, bias=b2, scale=sc)
                with cmp.Else():
                    nc.scalar.activation(out=fb, in_=o_full[:, 384:768],
                                         func=AF.Identity, bias=b2, scale=sc)

                # ---------- MLP ----------
                if i > 0:
                    j = i - 1

                    def mlp_layer(lw, bias_ap, xin, width, tag):
                        t = wide.tile([128, width], BF16, tag=tag)
                        for h0 in range(0, width, 384):
                            wdt = min(384, width - h0)
                            pp = ps_a.tile([128, 512], F32, tag="pa")
                            nc.tensor.matmul(pp[:, 0:wdt], lw,
                                             xin[:, h0:h0 + wdt],
                                             start=True, stop=True)
                            nc.scalar.activation(out=t[:, h0:h0 + wdt],
                                                 in_=pp[:, 0:wdt], func=AF.Relu,
                                                 bias=bias_ap, scale=1.0)
                        return t

                    y1f = mlp_layer(W['em1'][j], W['emb1'][j], f_full, 768, "y1f")
                    y2f = mlp_layer(W['em2'][j], W['emb2'][j], y1f, 768, "y2f")
                    o2f = wide.tile([128, 768], F32, tag="o2f")
                    nc.vector.tensor_tensor(out=o2f, in0=f_full, in1=y2f, op=OP.add)
                    y1q = mlp_layer(W['em1'][j], W['emb1'][j], fq, 96, "y1q")
                    y2q = mlp_layer(W['em2'][j], W['emb2'][j], y1q, 96, "y2q")
                    o2q = wide.tile([128, 96], F32, tag="o2q")
                    nc.vector.tensor_tensor(out=o2q, in0=fq, in1=y2q, op=OP.add)

                    st2 = smalls.tile([128, 2, 6], F32, tag="st2")
                    nc.vector.bn_stats(out=st2[:, 0, :], in_=o2f[:, 0:384])
                    nc.vector.bn_stats(out=st2[:, 1, :], in_=o2f[:, 384:768])
                    mv2 = smalls.tile([128, 2], F32, tag="mv2")
                    nc.vector.bn_aggr(out=mv2, in_=st2)
                    lnv2 = smalls.tile([128, 1], F32, tag="lnv")
                    nc.scalar.activation(out=lnv2, in_=mv2[:, 1:2], func=AF.Ln,
                                         bias=eps_t, scale=1.0)
                    rs2 = smalls.tile([128, 1], F32, tag="rs")
                    nc.scalar.activation(out=rs2, in_=lnv2, func=AF.Exp, bias=0.0,
                                         scale=-0.5)
                    sc2 = smalls.tile([128, 1], F32, tag="sc")
                    nc.vector.tensor_tensor(out=sc2, in0=W['emg'][j], in1=rs2,
                                            op=OP.mult)
                    b22 = smalls.tile([128, 1], F32, tag="b2")
                    nc.vector.tensor_scalar(out=b22, in0=mv2[:, 0:1], scalar1=sc2,
                                            scalar2=None, op0=OP.mult)
                    nc.vector.tensor_tensor(out=b22, in0=W['embe'][j], in1=b22,
                                            op=OP.subtract)
                    f_full = affine_evict(o2f[:], sc2, b22, [128, 768], tag="ffull")
                    fb = fpool.tile([128, 384], BF16, tag="fb")
                    with tc.If(pid < 4) as cmp:
                        nc.scalar.activation(out=fb, in_=o2f[:, 0:384],
                                             func=AF.Identity, bias=b22, scale=sc2)
                    with cmp.Else():
                        nc.scalar.activation(out=fb, in_=o2f[:, 384:768],
                                             func=AF.Identity, bias=b22, scale=sc2)
                    fq = fpool.tile([128, 96], BF16, tag="fq")
                    nc.vector.tensor_scalar(out=fq, in0=o2q, scalar1=sc2,
                                            scalar2=b22, op0=OP.mult, op1=OP.add)

            # ---------- final FC + max ----------
            for bb in range(2):
                fbb = f_full[:, bb * 384:(bb + 1) * 384]
                e1 = []
                for h in range(2):
                    pp = ps_a.tile([128, 512], F32, tag="pa")
                    nc.tensor.matmul(pp[:, 0:384], W['f1'][h], fbb, start=True,
                                     stop=True)
                    e1t = wide.tile([128, 384], BF16, tag=f"e1{h}")
                    nc.scalar.activation(out=e1t, in_=pp[:, 0:384], func=AF.Relu,
                                         bias=W['f1b'][h], scale=1.0)
                    e1.append(e1t)
                for h in range(2):
                    pp = ps_b.tile([128, 512], F32, tag="pb")
                    nc.tensor.matmul(pp[:, 0:384], W['f2'][h][0], e1[0], start=True,
                                     stop=False)
                    nc.tensor.matmul(pp[:, 0:384], W['f2'][h][1], e1[1], start=False,
                                     stop=True)
                    mx = smalls.tile([128, 1], F32, tag="mx")
                    nc.vector.tensor_reduce(out=mx, in_=pp[:, 0:384],
                                            axis=mybir.AxisListType.X, op=OP.max)
                    ot = smalls.tile([128, 1], F32, tag="ot")
                    nc.vector.tensor_scalar(out=ot, in0=mx, scalar1=W['f2b'][h],
                                            scalar2=None, op0=OP.add)
                    nc.sync.dma_start(
                        out=out_d[bb:bb + 1, h * 128:(h + 1) * 128], in_=ot)

    nc.compile()
    _CACHE[variant] = nc
    return nc


def _prep_inputs(inputs):
    """Host-side constant relayout + per-core packing. Returns in_maps list."""
    xyz = _f32(inputs["xyz"])          # [2, 384, 3]
    feats = _f32(inputs["feats"])      # [2, 384, 1]

    Wq, Wk, Wv = inputs["tb_Wq"], inputs["tb_Wk"], inputs["tb_Wv"]
    Wg1, bg1 = inputs["tb_Wg1"], inputs["tb_bg1"]
    Wg2, bg2 = inputs["tb_Wg2"], inputs["tb_bg2"]
    Wpe, bpe = inputs["tb_Wpe"], inputs["tb_bpe"]

    wpack = np.zeros((128, WCOLS), np.float32)
    vpack = np.zeros((128, VCOLS), np.float32)
    rpack_c = np.zeros((3, RCOLS), np.float32)   # per-core cols filled later
    qpack_c = np.zeros((1, QCOLS), np.float32)

    for i in range(NB):
        Ws, Wd = _wpe_split(_f32(Wpe[i]))
        g1 = _f32(Wg1[i])
        wpack[:, W_WG1 + 128 * i: W_WG1 + 128 * (i + 1)] = g1.T
        wpack[:, W_WG2 + 128 * i: W_WG2 + 128 * (i + 1)] = _f32(Wg2[i]).T
        wpack[:, W_NWK + 128 * i: W_NWK + 128 * (i + 1)] = (-_f32(Wk[i])).T
        wpack[:, W_WV + 128 * i: W_WV + 128 * (i + 1)] = _f32(Wv[i]).T
        wpack[:, W_G1Q + 128 * i: W_G1Q + 128 * (i + 1)] = (g1 @ _f32(Wq[i])).T
        wpack[:, W_G1P + 128 * i: W_G1P + 128 * (i + 1)] = _rep3((g1 @ Ws).T)
        wpack[:, W_PS + 128 * i: W_PS + 128 * (i + 1)] = _rep3(Ws.T)
        rpack_c[:, R_NPD4 + 128 * i: R_NPD4 + 128 * (i + 1)] = (-4.0 * Wd).T
        rpack_c[:, R_G1PD4 + 128 * i: R_G1PD4 + 128 * (i + 1)] = (4.0 * (g1 @ Wd)).T
        rpack_c[:, R_PD4 + 128 * i: R_PD4 + 128 * (i + 1)] = (4.0 * Wd).T
        qpack_c[0, Q_C1 + 128 * i: Q_C1 + 128 * (i + 1)] = g1 @ _f32(bpe[i]) + _f32(bg1[i])
        qpack_c[0, Q_BPE + 128 * i: Q_BPE + 128 * (i + 1)] = _f32(bpe[i])
        vpack[:, V_BG2 + i] = _f32(bg2[i])
        vpack[:, V_GAM + i] = _f32(inputs["tb_gamma"][i])
        vpack[:, V_BET + i] = _f32(inputs["tb_beta"][i])

    wpack[:, W_I128: W_I128 + 128] = np.eye(128, dtype=np.float32)
    for j in range(NF):
        wpack[:, W_EM1 + 128 * j: W_EM1 + 128 * (j + 1)] = _f32(inputs["em_W1"][j]).T
        wpack[:, W_EM2 + 128 * j: W_EM2 + 128 * (j + 1)] = _f32(inputs["em_W2"][j]).T
        vpack[:, V_EMB1 + j] = _f32(inputs["em_b1"][j])
        vpack[:, V_EMB2 + j] = _f32(inputs["em_b2"][j])
        vpack[:, V_EMG + j] = _f32(inputs["em_gamma"][j])
        vpack[:, V_EMBE + j] = _f32(inputs["em_beta"][j])
    W1T = _f32(inputs["fcf_W1"]).T           # [128, 256]
    for h in range(2):
        wpack[:, W_F1 + 128 * h: W_F1 + 128 * (h + 1)] = W1T[:, h * 128:(h + 1) * 128]
        vpack[:, V_F1B + h] = _f32(inputs["fcf_b1"])[h * 128:(h + 1) * 128]
        vpack[:, V_F2B + h] = _f32(inputs["fcf_b2"])[h * 128:(h + 1) * 128]
    W2T = _f32(inputs["fcf_W2"]).T           # [256, 256]
    for h in range(2):
        for k in range(2):
            wpack[:, W_F2 + 128 * (2 * h + k): W_F2 + 128 * (2 * h + k + 1)] = \
                W2T[k * 128:(k + 1) * 128, h * 128:(h + 1) * 128]
    vpack[:, V_ENCB] = _f32(inputs["enc_b"])
    qpack_c[0, Q_FEATS: Q_FEATS + 768] = feats.reshape(768)
    qpack_c[0, Q_ENC: Q_ENC + 128] = _f32(inputs["enc_W"])[:, 0]

    wpack_b = _bf(wpack)
    vpack_f = _f32(vpack)

    in_maps = []
    for c in range(8):
        b, qo = c // 4, (c % 4) * 96
        xk = xyz[b].T                      # [3, 384]
        S4 = np.zeros((4, NCH, 96), np.float32)
        for cch in range(NCH):
            for qr in range(3):
                qg = qo + 3 * cch + qr
                for j in range(3):
                    for t in range(10):
                        col = 32 * qr + 10 * j + t
                        s = np.float32(4.0 * FREQS[t % 5] / TWO_PI)
                        off = np.float32(0.25 if t >= 5 else 0.0)
                        S4[j, cch, col] = s
                        S4[3, cch, col] = off - s * np.float32(xyz[b, qg, j])
        fpack = np.zeros((4, FCOLS), np.float32)
        fpack[:, F_S4: F_S4 + NCH * 96] = S4.reshape(4, NCH * 96)
        fpack[0:3, F_XKO: F_XKO + 384] = xk
        fpack[3, F_XKO: F_XKO + 384] = 1.0

        rpack = rpack_c.copy()
        rpack[:, R_XKB: R_XKB + 384] = xk
        rpack[:, R_XQB: R_XQB + 96] = xk[:, qo:qo + 96]
        qpack = qpack_c.copy()
        qpack[0, Q_FB: Q_FB + 384] = feats[b].reshape(384)
        qpack[0, Q_FQ: Q_FQ + 96] = feats[b, qo:qo + 96].reshape(96)

        in_maps.append({
            "wpack": wpack_b,
            "vpack": vpack_f,
            "rpack": _bf(rpack),
            "qpack": _bf(qpack),
            "fpack": _f32(fpack),
        })
    return in_maps


def kernel(**inputs):
    from concourse.bass_utils import run_bass_kernel_spmd

    nc = _build()
    in_maps = _prep_inputs(inputs)
    res = run_bass_kernel_spmd(nc, in_maps, list(range(8)))
    return np.asarray(res.results[0]["out"], np.float32)


if __name__ == "__main__":
    rng = np.random.RandomState(0)
    fake = {
        "xyz": rng.randn(2, 384, 3).astype(np.float32),
        "feats": rng.randn(2, 384, 1).astype(np.float32),
    }
    print("smoke build only")
